# revision 29
# baseline (speedup 1.0000x reference)
"""Trainium2 Bass kernel for nn_CXINGeneral_1425929142863 (GNN message passing).

Math (per branch b, with epsilon=0):
    agg_b  = A_b @ x_src_b              (sparse gather + segment-sum, IN_CH space)
    h_b    = relu-MLP_b( agg_b @ W_b + x_target )     (3 layers)
    out    = concat(h0, h1) @ Wm + bm

Key rewrite: A @ (x_src @ W) == (A @ x_src) @ W — aggregate in IN_CH=128
space first, so every dense matmul is local to the target shard.

Distribution: target rows sharded 8 ways (6250 rows/core); edge lists
partitioned host-side by target-row ownership and sorted by local row block.
The host materializes each edge's source feature row (pure indexing, no
arithmetic) into a chunk-packed stream, so the device needs no indirect DMA
(the SWDGE gather path, 994 ns fixed cost per indirect DMA on the Pool
engine, was the original bottleneck).  The scatter one-hot S (vals placed at
local-row offsets) is streamed from HBM as bf16.

Device pipeline per core (all matmuls bf16, fp32 PSUM), fully interleaved at
512-row window granularity so DMA, TensorE, ScalarE and VectorE overlap:

  loop j over 13 windows:   scatter blocks 4j..4j+3 of branch 0
                            dense window j of branch 0
  loop j over 13 windows:   scatter blocks 4j..4j+3 of branch 1
                            dense window j of branch 1
                            merge window j  (transposed, Wm stationary)

  - scatter block: X [128e, kb*128ch] and S [128e, kb*128r] arrive in one
    batched DMA per 4-block group; TensorE accumulates X_k.T @ S_k in PSUM;
    ScalarE copies the PSUM block to aggT (bf16).  kb varies per block
    (tight packing, zero-padded lanes only in the last chunk of a block).
  - dense window: head matmul + x_target add (VectorE; x_target resident),
    3x (matmul + fused bias+relu, split ScalarE/VectorE by output half).
  - merge stays transposed ([256, rows] output, host transposes back).
"""

import sys
import types

import numpy as np
import ml_dtypes

import concourse.bass as bass
import concourse.mybir as mybir
import concourse.tile as tile
from concourse import bacc
import concourse.bass_utils as bass_utils
from concourse.bass_utils import run_bass_kernel_spmd

F32 = mybir.dt.float32
BF16 = mybir.dt.bfloat16
I32 = mybir.dt.int32

NP_BF16 = ml_dtypes.bfloat16


def _install_profile_hook():
    """This container's antenv lacks axon_hooks; reconstruct so trace=True works."""
    try:
        import antenv.axon_hooks  # noqa: F401
        return
    except ImportError:
        pass
    try:
        from trn_agent_boot.trn_boot import _ntff_profile_via_ctypes
    except ImportError:
        return
    mod = types.ModuleType("antenv.axon_hooks")
    hook = _ntff_profile_via_ctypes("/opt/axon/libaxon_pjrt.so")
    mod.get_axon_ntff_profile_hook = lambda: hook
    sys.modules["antenv.axon_hooks"] = mod
    bass_utils.upload_artifacts = lambda tmpdir: f"local:{tmpdir}"


class Cfg:
    def __init__(self, n_t=50000, n_s=100000, e=400000, n_cores=8):
        self.N_T = n_t
        self.N_S = n_s
        self.E = e
        self.NC = n_cores
        self.IN_CH = 128
        self.OUT_CH = 256
        self.N_MLP = 3
        self.NT_LOC = n_t // n_cores          # 6250
        self.R = 64                            # scatter row-block width
        self.NBLK = -(-self.NT_LOC // self.R)  # 98
        self.WIN = 512                         # dense row-window width


CFG = Cfg()


# ----------------------------------------------------------------- host prep

def _prep_edges(cfg, rows, cols, vals, x_src_bf):
    """Partition + sort one branch's edges; build per-core packed streams.

    Chunks are tightly packed: block blk owns ceil(max_core_count(blk)/128)
    chunks (the same count on every core, so the SPMD program is uniform).

    Returns (x_arr [NC,128,C*128] bf16, s_arr [NC,128,C*R] bf16,
             kbs: list[49] chunks per block).
    """
    rows = np.asarray(rows, np.int64)
    cols = np.asarray(cols, np.int32)
    vals = np.asarray(vals, np.float32)

    core = rows // cfg.NT_LOC
    lrow = rows % cfg.NT_LOC
    blk = lrow // cfg.R
    d = lrow % cfg.R

    group = core * cfg.NBLK + blk             # global (core, block) id
    order = np.argsort(group, kind="stable")
    g_sorted = group[order]

    n_groups = cfg.NC * cfg.NBLK
    counts = np.bincount(g_sorted, minlength=n_groups)
    # per-block chunk count = max over cores (uniform SPMD program)
    kbs = [int((counts.reshape(cfg.NC, cfg.NBLK)[:, i].max() + 127) // 128)
           for i in range(cfg.NBLK)]
    kbs = [max(k, 1) for k in kbs]
    cstart = np.zeros(cfg.NBLK + 1, np.int64)
    np.cumsum(kbs, out=cstart[1:])
    C = int(cstart[-1])

    # rank of each edge within its group
    starts = np.zeros(n_groups, np.int64)
    np.cumsum(counts[:-1], out=starts[1:])
    rank = np.arange(len(rows)) - starts[g_sorted]

    core_s = core[order]
    blk_s = blk[order]
    d_s = d[order]
    cols_s = cols[order]
    vals_s = vals[order]

    chunk = cstart[blk_s] + rank // 128        # chunk id within core
    lane = rank % 128

    # packed per-edge source features: pure indexing, no arithmetic
    x_arr = np.zeros((cfg.NC, 128, C, 128), NP_BF16)
    x_arr[core_s, lane, chunk] = x_src_bf[cols_s]
    x_arr = x_arr.reshape(cfg.NC, 128, C * 128)

    s_arr = np.zeros((cfg.NC, 128, C * cfg.R), NP_BF16)
    s_arr[core_s, lane, chunk * cfg.R + d_s] = vals_s.astype(NP_BF16)
    return x_arr, s_arr, kbs


def prep_inputs(cfg, inputs):
    """Build the full list of per-core in_maps + the compile-time chunk counts."""
    x_target = np.asarray(inputs["x_target"], np.float32)
    xs_bf = [np.asarray(inputs[f"x_src{b}"], np.float32).astype(NP_BF16)
             for b in (0, 1)]

    xa0, sa0, kbs0 = _prep_edges(cfg, inputs["rows0"], inputs["cols0"],
                                 inputs["vals0"], xs_bf[0])
    xa1, sa1, kbs1 = _prep_edges(cfg, inputs["rows1"], inputs["cols1"],
                                 inputs["vals1"], xs_bf[1])

    W0 = np.asarray(inputs["W0"], np.float32)
    W1 = np.asarray(inputs["W1"], np.float32)
    w01 = np.ascontiguousarray(np.concatenate([W0, W1], axis=1)).astype(NP_BF16)

    mlpw = []
    for b in (0, 1):
        mw = np.asarray(inputs[f"mlp_W{b}"], np.float32)  # [3, 256, 256]
        blocks = []
        for l in range(cfg.N_MLP):
            for icb in range(2):
                for ocb in range(2):
                    blocks.append(mw[l, icb * 128:(icb + 1) * 128, ocb * 128:(ocb + 1) * 128])
        mlpw.append(np.ascontiguousarray(np.concatenate(blocks, axis=1)).astype(NP_BF16))

    mlpb = []
    for b in (0, 1):
        mb_ = np.asarray(inputs[f"mlp_b{b}"], np.float32)  # [3, 256]
        cols_ = []
        for l in range(cfg.N_MLP):
            for ocb in range(2):
                cols_.append(mb_[l, ocb * 128:(ocb + 1) * 128][:, None])
        mlpb.append(np.ascontiguousarray(np.concatenate(cols_, axis=1)))  # [128, 6] f32

    Wm = np.asarray(inputs["Wm"], np.float32)  # [512, 256]
    wm = np.ascontiguousarray(
        np.concatenate([Wm[i * 128:(i + 1) * 128, :] for i in range(4)], axis=1)
    ).astype(NP_BF16)  # [128, 1024]
    bm = np.asarray(inputs["bm"], np.float32)
    bmcol = np.ascontiguousarray(np.stack([bm[:128], bm[128:]], axis=1))  # [128, 2]

    in_maps = []
    for c in range(cfg.NC):
        xt = np.ascontiguousarray(
            x_target[c * cfg.NT_LOC:(c + 1) * cfg.NT_LOC].T).astype(NP_BF16)
        in_maps.append({
            "xp0": np.ascontiguousarray(xa0[c]),
            "xp1": np.ascontiguousarray(xa1[c]),
            "s0": np.ascontiguousarray(sa0[c]),
            "s1": np.ascontiguousarray(sa1[c]),
            "xt": xt,
            "w01": w01, "mlpw0": mlpw[0], "mlpw1": mlpw[1],
            "b0": mlpb[0], "b1": mlpb[1],
            "wm": wm, "bmcol": bmcol,
        })
    return in_maps, (tuple(kbs0), tuple(kbs1))


# ------------------------------------------------------------------- builder

def build(cfg, kbs):
    """Build the SPMD Bass program. kbs = (kbs0, kbs1) chunks per row block."""
    nc = bacc.Bacc("TRN2", target_bir_lowering=False, debug=False)

    cstart = []
    for b in (0, 1):
        cs = [0]
        for k in kbs[b]:
            cs.append(cs[-1] + k)
        cstart.append(cs)
    C = [cstart[0][-1], cstart[1][-1]]

    xpack = [nc.declare_dram_parameter(f"xp{b}", [128, C[b] * 128], BF16, isOutput=False)
             for b in (0, 1)]
    sd = [nc.declare_dram_parameter(f"s{b}", [128, C[b] * cfg.R], BF16, isOutput=False)
          for b in (0, 1)]
    xt_d = nc.declare_dram_parameter("xt", [cfg.OUT_CH, cfg.NT_LOC], BF16, isOutput=False)
    w01_d = nc.declare_dram_parameter("w01", [128, 512], BF16, isOutput=False)
    mlpw_d = [nc.declare_dram_parameter(f"mlpw{b}", [128, cfg.N_MLP * 4 * 128], BF16,
                                        isOutput=False) for b in (0, 1)]
    b_d = [nc.declare_dram_parameter(f"b{b}", [128, cfg.N_MLP * 2], F32, isOutput=False)
           for b in (0, 1)]
    wm_d = nc.declare_dram_parameter("wm", [128, 4 * cfg.OUT_CH], BF16, isOutput=False)
    bm_d = nc.declare_dram_parameter("bmcol", [128, 2], F32, isOutput=False)
    out_d = nc.declare_dram_parameter("out", [cfg.OUT_CH, cfg.NT_LOC], BF16, isOutput=True)

    AG = cfg.NBLK * cfg.R  # aggT free width (>= NT_LOC)

    # dense row windows; window j covers scatter blocks 4j..4j+3
    wins = []
    w0 = 0
    while w0 < cfg.NT_LOC:
        wins.append((w0, min(cfg.WIN, cfg.NT_LOC - w0)))
        w0 += cfg.WIN
    BPW = cfg.WIN // cfg.R  # blocks per window = 4

    with tile.TileContext(nc) as tc:
        with (
            tc.tile_pool(name="wpool", bufs=1) as wpool,
            tc.tile_pool(name="hbig", bufs=1) as hbig,
            tc.tile_pool(name="xep", bufs=3) as xep,
            tc.tile_pool(name="spool", bufs=3) as spool,
            tc.tile_pool(name="hwin", bufs=2) as hwin,
            tc.tile_pool(name="outp", bufs=3) as outp,
            tc.tile_pool(name="pscat", bufs=4, space="PSUM") as pscat,
            tc.tile_pool(name="pdense", bufs=4, space="PSUM") as pdense,
        ):
            # --- resident weights + x_target, issued on the scalar queue so
            # the scatter streams own the sync queue.  Ordered so the data
            # needed first transfers first: branch-0 weights, the first two
            # x_target windows, branch-1 + merge weights, remaining windows.
            w01_sb = wpool.tile([128, 512], BF16, tag="w01")
            mlpw_sb = [wpool.tile([128, cfg.N_MLP * 4 * 128], BF16,
                                  tag=f"mlpw{b}", name=f"mlpw{b}")
                       for b in (0, 1)]
            b_sb = [wpool.tile([128, cfg.N_MLP * 2], F32, tag=f"b{b}", name=f"b{b}")
                    for b in (0, 1)]
            wm_sb = wpool.tile([128, 4 * cfg.OUT_CH], BF16, tag="wm")
            bm_sb = wpool.tile([128, 2], F32, tag="bmcol")
            # x_target resident, but loaded in window-sized slices so the
            # early windows' data isn't queued behind 3.2MB of transfers
            xt_sb = [wpool.tile([128, cfg.NT_LOC], BF16, tag=f"xt{ocb}",
                                name=f"xt{ocb}")
                     for ocb in range(2)]

            def xt_load(j, w0, wl):
                for ocb in range(2):
                    nc.sync.dma_start(
                        out=xt_sb[ocb][:, w0:w0 + wl],
                        in_=xt_d[ocb * 128:(ocb + 1) * 128, w0:w0 + wl])

            nc.scalar.dma_start(out=w01_sb[:], in_=w01_d[:])
            nc.scalar.dma_start(out=mlpw_sb[0][:], in_=mlpw_d[0][:])
            nc.scalar.dma_start(out=b_sb[0][:], in_=b_d[0][:])

            # --- persistent activations
            aggT = [hbig.tile([128, AG], BF16, tag="agg", name=f"agg{b}") for b in (0, 1)]
            hT = [[hbig.tile([128, cfg.NT_LOC], BF16, tag=f"h{b}{half}", name=f"h{b}{half}")
                   for half in (0, 1)] for b in (0, 1)]

            def scatter_group(b, j):
                """Stream + matmul scatter blocks [BPW*j, BPW*(j+1)) of branch b."""
                blo = BPW * j
                bhi = min(BPW * (j + 1), cfg.NBLK)
                c0, c1 = cstart[b][blo], cstart[b][bhi]
                nch = c1 - c0
                if nch == 0:
                    return
                xg = xep.tile([128, nch * 128], BF16, tag="xg")
                nc.sync.dma_start(
                    out=xg[:], in_=xpack[b][:, c0 * 128:c1 * 128])
                sg = spool.tile([128, nch * cfg.R], BF16, tag="sg")
                nc.sync.dma_start(
                    out=sg[:], in_=sd[b][:, c0 * cfg.R:c1 * cfg.R])
                for blk in range(blo, bhi):
                    kb = kbs[b][blk]
                    psum = pscat.tile([128, cfg.R], F32, tag="ps")
                    for k in range(kb):
                        c = cstart[b][blk] - c0 + k   # chunk offset inside group
                        nc.tensor.matmul(
                            out=psum[:], lhsT=xg[:, c * 128:(c + 1) * 128],
                            rhs=sg[:, c * cfg.R:(c + 1) * cfg.R],
                            start=(k == 0), stop=(k == kb - 1))
                    agg_ap = aggT[b][:, blk * cfg.R:(blk + 1) * cfg.R]
                    if blk % 2 == 0:
                        nc.vector.tensor_copy(out=agg_ap, in_=psum[:])
                    else:
                        nc.scalar.copy(out=agg_ap, in_=psum[:])

            def dense_pair(b, jlist):
                """Head + MLP for windows jlist of branch b -> hT[b].

                The two windows' matmuls are interleaved so TensorE has the
                other window's work while one window waits for its
                activations to come back from ScalarE/VectorE.
                """
                cur = {}
                for si, j in enumerate(jlist):
                    w0, wl = wins[j]
                    for ocb in range(2):
                        ph = pdense.tile([128, cfg.WIN], F32, tag="pd")
                        nc.tensor.matmul(
                            out=ph[:, :wl],
                            lhsT=w01_sb[:, b * 256 + ocb * 128: b * 256 + ocb * 128 + 128],
                            rhs=aggT[b][:, w0:w0 + wl],
                            start=True, stop=True)
                        h = hwin.tile([128, cfg.WIN], BF16, tag=f"hin{ocb}{si}")
                        nc.vector.tensor_add(
                            out=h[:, :wl], in0=ph[:, :wl], in1=xt_sb[ocb][:, w0:w0 + wl])
                        cur[(si, ocb)] = h
                for l in range(cfg.N_MLP):
                    nxt = {}
                    for si, j in enumerate(jlist):
                        w0, wl = wins[j]
                        for ocb in range(2):
                            pm = pdense.tile([128, cfg.WIN], F32, tag="pd")
                            for icb in range(2):
                                nc.tensor.matmul(
                                    out=pm[:, :wl],
                                    lhsT=mlpw_sb[b][:, (l * 4 + icb * 2 + ocb) * 128:
                                                    (l * 4 + icb * 2 + ocb) * 128 + 128],
                                    rhs=cur[(si, icb)][:, :wl],
                                    start=(icb == 0), stop=(icb == 1))
                            if l == cfg.N_MLP - 1:
                                hn_ap = hT[b][ocb][:, w0:w0 + wl]
                            else:
                                hn = hwin.tile([128, cfg.WIN], BF16, tag=f"h{l}{ocb}{si}")
                                hn_ap = hn[:, :wl]
                            bias_ap = b_sb[b][:, l * 2 + ocb: l * 2 + ocb + 1]
                            if ocb == 1 and l == 1:
                                nc.vector.tensor_scalar(
                                    out=hn_ap, in0=pm[:, :wl],
                                    scalar1=bias_ap, scalar2=0.0,
                                    op0=mybir.AluOpType.add,
                                    op1=mybir.AluOpType.max)
                            else:
                                nc.scalar.activation(
                                    out=hn_ap, in_=pm[:, :wl],
                                    func=mybir.ActivationFunctionType.Relu,
                                    bias=bias_ap, scale=1.0)
                            if l != cfg.N_MLP - 1:
                                nxt[(si, ocb)] = hn
                    if l != cfg.N_MLP - 1:
                        cur = nxt

            def merge_pair(jlist):
                """outT[oc, windows] = Wm.T @ concat(h0,h1) + bm."""
                for j in jlist:
                    w0, wl = wins[j]
                    for ocb in range(2):
                        po = pdense.tile([128, cfg.WIN], F32, tag="pd")
                        for ic in range(4):
                            nc.tensor.matmul(
                                out=po[:, :wl],
                                lhsT=wm_sb[:, ic * 256 + ocb * 128: ic * 256 + ocb * 128 + 128],
                                rhs=hT[ic // 2][ic % 2][:, w0:w0 + wl],
                                start=(ic == 0), stop=(ic == 3))
                        o_sb = outp.tile([128, cfg.WIN], BF16, tag="o")
                        nc.scalar.activation(
                            out=o_sb[:, :wl], in_=po[:, :wl],
                            func=mybir.ActivationFunctionType.Identity,
                            bias=bm_sb[:, ocb:ocb + 1], scale=1.0)
                        nc.sync.dma_start(
                            out=out_d[ocb * 128:(ocb + 1) * 128, w0:w0 + wl],
                            in_=o_sb[:, :wl])

            # software-pipelined emission: scatter group i is queued ahead of
            # dense window i-1 so TensorE always has scatter matmuls to chew
            # while the cross-engine (copy/act) results for the dense window
            # land; merge windows lag their dense window by one stage.
            nw = len(wins)
            # remaining weights on the scalar queue (needed from branch 1 /
            # merge onward, well after the pipeline has started)
            nc.scalar.dma_start(out=mlpw_sb[1][:], in_=mlpw_d[1][:])
            nc.scalar.dma_start(out=b_sb[1][:], in_=b_d[1][:])
            nc.scalar.dma_start(out=wm_sb[:], in_=wm_d[:])
            nc.scalar.dma_start(out=bm_sb[:], in_=bm_d[:])

            # pipeline in steps of two windows: scatter groups for step i,
            # dense pair for step i-1, merge pair (branch 1) for step i-2
            pairs = [list(range(j, min(j + 2, nw))) for j in range(0, nw, 2)]
            steps = [(0, p) for p in pairs] + [(1, p) for p in pairs]
            for i in range(len(steps) + 2):
                if i < len(steps):
                    sb_, sp = steps[i]
                    for j in sp:
                        scatter_group(sb_, j)
                        if sb_ == 0:
                            # x_target windows ride the sync queue just
                            # behind their scatter group's streams
                            xt_load(j, *wins[j])
                if 1 <= i <= len(steps):
                    db_, dp = steps[i - 1]
                    dense_pair(db_, dp)
                if i >= 2:
                    mb_, mp = steps[i - 2]
                    if mb_ == 1:
                        merge_pair(mp)

    nc.compile()
    return nc


# -------------------------------------------------------------------- runner

_CACHE = {}


def kernel(**inputs) -> np.ndarray:
    _install_profile_hook()
    cfg = CFG
    in_maps, kbs = prep_inputs(cfg, inputs)
    key = ("v10", kbs)
    if key not in _CACHE:
        _CACHE[key] = build(cfg, kbs)
    nc = _CACHE[key]
    trace = bool(int(__import__("os").environ.get("KERNEL_TRACE", "0")))
    r = run_bass_kernel_spmd(nc, in_maps, core_ids=list(range(cfg.NC)), trace=trace)
    kernel.last_result = r
    out = np.concatenate(
        [np.asarray(r.results[c]["out"], np.float32).T for c in range(cfg.NC)], axis=0)
    return np.ascontiguousarray(out, np.float32)


kernel.last_result = None
